# revision 6
# baseline (speedup 1.0000x reference)
"""Causal self-attention (B=4, T=2048, C=1024, NH=16) on 8 TRN2 NeuronCores.

Sharding (per spec hint): tensor-parallel over heads x data-parallel over batch.
Core i handles batch b = i//2 and head-group g = i%2 (8 heads each).
  - c_attn column-parallel: each core computes q,k,v for its 8 heads.
  - attention: fully local per core (its heads, its batch element).
  - c_proj row-parallel: each core computes a partial (T,C) output from its
    512 features; a 2-core ReduceScatter over pairs [[0,1],[2,3],[4,5],[6,7]]
    sums the partials, each core keeping half the rows. Host concatenates.

Device algorithm (per core), all matmuls bf16 with fp32 PSUM accumulation:
  xT (C,T) staged transposed by host.
  qT = wq^T @ xT, kT = wk^T @ xT   (feature-major, 4 chunks of 128)
  v  = x @ wv                      (token-major) + ones column per head
  per head pair (2fc, 2fc+1), per q-block Q (512 wide):
    s^T[kchunk] = kT_h^T @ qT_h    (K=64 contraction, row-tiled pair -> concurrent)
    p = exp(0.125 * s^T)  (ScalarE, bf16 out); causal-zeroed on GpSimd for
        diagonal chunks; fully-masked chunks skipped entirely.
    o^T[65,512] += v_aug_h^T @ p   (v_aug has a ones column -> row 64 = softmax
        denominators, fused into the same matmul)
    yT_h = o^T[0:64] * (1/o^T[64])  (PE K=1 broadcast of the reciprocal row)
  partial[T-block] = yT^T @ wp + 0.5*b_proj ; ReduceScatter(add) over the pair.
"""

import sys

if "/opt/trn_rl_repo" not in sys.path:
    sys.path.insert(0, "/opt/trn_rl_repo")

import numpy as np
import ml_dtypes

import concourse.bass as bass
import concourse.bacc as bacc
import concourse.mybir as mybir
import concourse.tile as tile
from concourse.bass import ts, ds
from concourse.bass_utils import run_bass_kernel_spmd

BF16 = ml_dtypes.bfloat16
N_CORES = 8
B, T, C = 4, 2048, 1024
NH, HS = 16, 64
H_LOC = NH // 2        # heads per core
F = H_LOC * HS         # 512 local qkv features
NFC = F // 128         # 4 feature chunks (one head pair each)
NKC = T // 128         # 16 key chunks
NQ = T // 512          # 4 query blocks
NCOL = C // 512        # 2 output column blocks
REPLICA_GROUPS = [[0, 1], [2, 3], [4, 5], [6, 7]]

FP32 = mybir.dt.float32
BF = mybir.dt.bfloat16


def _build_nc():
    # Bacc (not plain Bass): its compile() pipeline runs
    # generate_event_semaphores, which splits sync waits so no instruction
    # carries more than the hardware allows (walrus rejects >1 otherwise).
    nc = bacc.Bacc(None, target_bir_lowering=False, num_devices=N_CORES)

    xT = nc.dram_tensor("xT", [C, T], BF, kind="ExternalInput")
    wq = nc.dram_tensor("wq", [C, F], BF, kind="ExternalInput")
    wk = nc.dram_tensor("wk", [C, F], BF, kind="ExternalInput")
    wv = nc.dram_tensor("wv", [C, F], BF, kind="ExternalInput")
    bq = nc.dram_tensor("bq", [F], FP32, kind="ExternalInput")
    bk = nc.dram_tensor("bk", [F], FP32, kind="ExternalInput")
    bv = nc.dram_tensor("bv", [F], FP32, kind="ExternalInput")
    wp = nc.dram_tensor("wp", [F, C], BF, kind="ExternalInput")
    bp = nc.dram_tensor("bp", [C], FP32, kind="ExternalInput")
    out = nc.dram_tensor("out", [T // 2, C], FP32, kind="ExternalOutput")

    with tile.TileContext(nc) as tc:
        _body(tc, xT, wq, wk, wv, bq, bk, bv, wp, bp, out)
    nc.compile()
    return nc


def _body(tc, xT, wq, wk, wv, bq, bk, bv, wp, bp, out):
    nc = tc.nc
    import contextlib

    ctx = contextlib.ExitStack()
    with ctx:
        wpool = ctx.enter_context(tc.tile_pool(name="weights", bufs=1))
        apool = ctx.enter_context(tc.tile_pool(name="acts", bufs=1))
        ppool = ctx.enter_context(tc.tile_pool(name="ptiles", bufs=3))
        npool = ctx.enter_context(tc.tile_pool(name="norm", bufs=2))
        outp = ctx.enter_context(tc.tile_pool(name="outsb", bufs=3))
        ps_s = ctx.enter_context(tc.tile_pool(name="ps_s", bufs=2, space="PSUM"))
        ps_o = ctx.enter_context(tc.tile_pool(name="ps_o", bufs=1, space="PSUM"))
        ps_bc = ctx.enter_context(tc.tile_pool(name="ps_bc", bufs=2, space="PSUM"))
        dpool = ctx.enter_context(tc.tile_pool(name="dram", bufs=1, space="DRAM"))

        # ---- stage inputs into SBUF ----
        x_sb = wpool.tile([128, C // 128, T], BF)
        nc.sync.dma_start(out=x_sb, in_=xT.rearrange("(ko p) t -> p ko t", p=128))
        wq_sb = wpool.tile([128, C // 128, F], BF)
        nc.sync.dma_start(out=wq_sb, in_=wq.rearrange("(ko p) f -> p ko f", p=128))
        wk_sb = wpool.tile([128, C // 128, F], BF)
        nc.sync.dma_start(out=wk_sb, in_=wk.rearrange("(ko p) f -> p ko f", p=128))
        wv_sb = wpool.tile([128, C // 128, F], BF)
        nc.sync.dma_start(out=wv_sb, in_=wv.rearrange("(ko p) f -> p ko f", p=128))
        wp_sb = wpool.tile([128, NFC, C], BF)
        nc.sync.dma_start(out=wp_sb, in_=wp.rearrange("(ko p) n -> p ko n", p=128))

        bq_sb = wpool.tile([128, NFC], FP32)
        nc.sync.dma_start(out=bq_sb, in_=bq.rearrange("(fo p) -> p fo", p=128))
        bk_sb = wpool.tile([128, NFC], FP32)
        nc.sync.dma_start(out=bk_sb, in_=bk.rearrange("(fo p) -> p fo", p=128))
        # broadcast biases across partitions (for token-major layouts)
        bv_bc = wpool.tile([128, F], FP32)
        nc.sync.dma_start(
            out=bv_bc,
            in_=bass.AP(tensor=bv.ap().tensor, offset=0, ap=[[0, 128], [1, F]]),
        )
        bp_bc = wpool.tile([128, C], FP32)
        nc.sync.dma_start(
            out=bp_bc,
            in_=bass.AP(tensor=bp.ap().tensor, offset=0, ap=[[0, 128], [1, C]]),
        )

        ones_sb = wpool.tile([1, 64], BF)
        nc.vector.memset(ones_sb, 1.0)

        # ---- persistent activations ----
        qT_sb = apool.tile([128, NFC, T], BF)   # q, feature-major
        kT_sb = apool.tile([128, NFC, T], BF)   # k, feature-major
        # v token-major, 66-stride per head: cols 0:64 = v, col 64 = ones
        v_sb = apool.tile([128, NKC, H_LOC, 66], BF)
        nc.vector.memset(v_sb[:, :, :, 64:65], 1.0)
        yT_sb = apool.tile([128, NFC, T], BF)   # attention out, feature-major

        partial = dpool.tile([T, C], FP32)      # c_proj partial (pre-reduce)
        rs_out = dpool.tile([T // 2, C], FP32)

        KO = C // 128  # 8 contraction chunks for the projections

        # ---- phase 1: qT, kT (feature-major) ----
        for name, w_sb, b_sb, dst in (("q", wq_sb, bq_sb, qT_sb), ("k", wk_sb, bk_sb, kT_sb)):
            for fc in range(NFC):
                for tq in range(NQ):
                    ps = ps_s.tile([128, 512], FP32, tag="sA")
                    for kc in range(KO):
                        nc.tensor.matmul(
                            ps,
                            lhsT=w_sb[:, kc, ts(fc, 128)],
                            rhs=x_sb[:, kc, ts(tq, 512)],
                            start=(kc == 0),
                            stop=(kc == KO - 1),
                        )
                    nc.scalar.activation(
                        out=dst[:, fc, ts(tq, 512)],
                        in_=ps,
                        func=mybir.ActivationFunctionType.Identity,
                        bias=b_sb[:, fc : fc + 1],
                        scale=1.0,
                    )

        # ---- phase 1b: v (token-major) ----
        for tc_i in range(NKC):
            ps = ps_s.tile([128, 512], FP32, tag="sB")
            for kc in range(KO):
                nc.tensor.matmul(
                    ps,
                    lhsT=x_sb[:, kc, ts(tc_i, 128)],
                    rhs=wv_sb[:, kc, :],
                    start=(kc == 0),
                    stop=(kc == KO - 1),
                )
            nc.vector.tensor_add(
                out=v_sb[:, tc_i, :, 0:64],
                in0=ps.rearrange("p (h f) -> p h f", h=H_LOC),
                in1=bv_bc.rearrange("p (h f) -> p h f", h=H_LOC),
            )

        # ---- phase 2+3: attention per q-block, then that block's c_proj ----
        for Q in range(NQ):
            nkc = 4 * Q + 4  # causal: only key chunks 0 .. 4Q+3 contribute
            for fc in range(NFC):  # head pair (2fc, 2fc+1)
                oA = ps_o.tile([65, 512], FP32, tag="oA")
                oB = ps_o.tile([65, 512], FP32, tag="oB")
                for kc in range(nkc):
                    diag = kc >= 4 * Q  # chunk crosses the causal boundary
                    sA = ps_s.tile([128, 512], FP32, tag="sA")
                    sB = ps_s.tile([128, 512], FP32, tag="sB")
                    nc.tensor.matmul(
                        sA,
                        lhsT=kT_sb[0:64, fc, ts(kc, 128)],
                        rhs=qT_sb[0:64, fc, ts(Q, 512)],
                        start=True,
                        stop=True,
                        tile_position=(0, 0),
                    )
                    nc.tensor.matmul(
                        sB,
                        lhsT=kT_sb[64:128, fc, ts(kc, 128)],
                        rhs=qT_sb[64:128, fc, ts(Q, 512)],
                        start=True,
                        stop=True,
                        tile_position=(64, 0),
                    )
                    pA = ppool.tile([128, 512], BF, tag="pA")
                    pB = ppool.tile([128, 512], BF, tag="pB")
                    nc.scalar.activation(
                        out=pA, in_=sA, func=mybir.ActivationFunctionType.Exp,
                        scale=0.125,
                    )
                    nc.scalar.activation(
                        out=pB, in_=sB, func=mybir.ActivationFunctionType.Exp,
                        scale=0.125,
                    )
                    if diag:
                        # zero entries with k_global > q_global:
                        # keep where 512*Q + qc - 128*kc - kr >= 0
                        for p_t in (pA, pB):
                            nc.gpsimd.affine_select(
                                out=p_t,
                                in_=p_t,
                                compare_op=mybir.AluOpType.is_ge,
                                fill=0.0,
                                base=512 * Q - 128 * kc,
                                channel_multiplier=-1,
                                pattern=[[1, 512]],
                            )
                    nc.tensor.matmul(
                        oA,
                        lhsT=v_sb[:, kc, 2 * fc, 0:65],
                        rhs=pA,
                        start=(kc == 0),
                        stop=(kc == nkc - 1),
                    )
                    nc.tensor.matmul(
                        oB,
                        lhsT=v_sb[:, kc, 2 * fc + 1, 0:65],
                        rhs=pB,
                        start=(kc == 0),
                        stop=(kc == nkc - 1),
                    )
                # normalize: yT_h = oT[0:64] * (1 / oT[64]) , broadcast via PE
                oA_sb = npool.tile([65, 512], FP32, tag="oAsb")
                oB_sb = npool.tile([65, 512], FP32, tag="oBsb")
                nc.vector.tensor_copy(out=oA_sb, in_=oA)
                nc.vector.tensor_copy(out=oB_sb, in_=oB)
                rA = npool.tile([1, 512], FP32, tag="rA")
                rB = npool.tile([1, 512], FP32, tag="rB")
                nc.vector.reciprocal(out=rA, in_=oA_sb[64:65, :])
                nc.vector.reciprocal(out=rB, in_=oB_sb[64:65, :])
                rA_bf = npool.tile([1, 512], BF, tag="rAbf")
                rB_bf = npool.tile([1, 512], BF, tag="rBbf")
                nc.vector.tensor_copy(out=rA_bf, in_=rA)
                nc.vector.tensor_copy(out=rB_bf, in_=rB)
                bcA = ps_bc.tile([64, 512], FP32, tag="bc")
                bcB = ps_bc.tile([64, 512], FP32, tag="bc")
                nc.tensor.matmul(bcA, lhsT=ones_sb, rhs=rA_bf, start=True, stop=True)
                nc.tensor.matmul(bcB, lhsT=ones_sb, rhs=rB_bf, start=True, stop=True)
                # head A lives on partitions 0:64 of chunk fc
                nc.vector.tensor_mul(
                    out=yT_sb[0:64, fc, ts(Q, 512)], in0=oA_sb[0:64, :], in1=bcA
                )
                # head B must land on partitions 64:128 -> stage + DMA shift
                yB = npool.tile([64, 512], BF, tag="yB")
                nc.vector.tensor_mul(out=yB, in0=oB_sb[0:64, :], in1=bcB)
                nc.sync.dma_start(out=yT_sb[64:128, fc, ts(Q, 512)], in_=yB)

            # c_proj for this block of 512 tokens
            for tb in range(4):
                trow = Q * 4 + tb
                for ncol in range(NCOL):
                    ps = ps_s.tile([128, 512], FP32, tag="sA")
                    for fc in range(NFC):
                        nc.tensor.matmul(
                            ps,
                            lhsT=yT_sb[:, fc, ts(trow, 128)],
                            rhs=wp_sb[:, fc, ts(ncol, 512)],
                            start=(fc == 0),
                            stop=(fc == NFC - 1),
                        )
                    o_sb = outp.tile([128, 512], FP32, tag="osb")
                    nc.vector.tensor_add(out=o_sb, in0=ps, in1=bp_bc[:, ts(ncol, 512)])
                    nc.sync.dma_start(
                        out=partial[ds(trow * 128, 128), ts(ncol, 512)], in_=o_sb
                    )

        # ---- reduce partials across the batch pair, keep our half ----
        nc.gpsimd.collective_compute(
            "ReduceScatter",
            mybir.AluOpType.add,
            replica_groups=REPLICA_GROUPS,
            ins=[partial[:]],
            outs=[rs_out[:]],
        )
        nc.sync.dma_start(out=out.ap(), in_=rs_out[:])


_NC_CACHE = None


def _get_nc():
    global _NC_CACHE
    if _NC_CACHE is None:
        _NC_CACHE = _build_nc()
    return _NC_CACHE


def kernel(x, w_attn, b_attn, w_proj, b_proj):
    x = np.asarray(x)
    w_attn = np.asarray(w_attn)
    b_attn = np.asarray(b_attn)
    w_proj = np.asarray(w_proj)
    b_proj = np.asarray(b_proj)

    nc = _get_nc()

    in_maps = []
    for i in range(N_CORES):
        b, g = i // 2, i % 2
        cols = slice(g * F, (g + 1) * F)
        in_maps.append(
            {
                "xT": np.ascontiguousarray(x[b].T).astype(BF16),
                "wq": np.ascontiguousarray(w_attn[:, g * F : (g + 1) * F]).astype(BF16),
                "wk": np.ascontiguousarray(
                    w_attn[:, C + g * F : C + (g + 1) * F]
                ).astype(BF16),
                "wv": np.ascontiguousarray(
                    w_attn[:, 2 * C + g * F : 2 * C + (g + 1) * F]
                ).astype(BF16),
                "bq": np.ascontiguousarray(b_attn[g * F : (g + 1) * F]).astype(
                    np.float32
                ),
                "bk": np.ascontiguousarray(b_attn[C + g * F : C + (g + 1) * F]).astype(
                    np.float32
                ),
                "bv": np.ascontiguousarray(
                    b_attn[2 * C + g * F : 2 * C + (g + 1) * F]
                ).astype(np.float32),
                "wp": np.ascontiguousarray(w_proj[g * F : (g + 1) * F, :]).astype(BF16),
                "bp": (b_proj * 0.5).astype(np.float32),
            }
        )

    global _last_in_maps
    _last_in_maps = in_maps  # stashed for external profiling harnesses
    res = run_bass_kernel_spmd(nc, in_maps, core_ids=list(range(N_CORES)))

    out = np.empty((B, T, C), dtype=np.float32)
    for b in range(B):
        out[b, : T // 2] = res.results[2 * b]["out"]
        out[b, T // 2 :] = res.results[2 * b + 1]["out"]
    return out


# revision 14
# speedup vs baseline: 1.3766x; 1.3766x over previous
"""Causal self-attention (B=4, T=2048, C=1024, NH=16) on 8 TRN2 NeuronCores.

Sharding (per spec hint): tensor-parallel over heads x data-parallel over batch.
Core i handles batch b = i//2 and head-group g = i%2 (8 heads each).
  - c_attn column-parallel: each core computes q,k,v for its 8 heads.
  - attention: fully local per core (its heads, its batch element).
  - c_proj row-parallel: each core computes a partial (T,C) output from its
    512 features; a 2-core ReduceScatter over pairs [[0,1],[2,3],[4,5],[6,7]]
    sums the partials, each core keeping half the rows. Host concatenates.

Device algorithm (per core), all matmuls bf16 with fp32 PSUM accumulation:
  xT (C,T) staged transposed by host.
  qT = wq^T @ xT, kT = wk^T @ xT   (feature-major, 4 chunks of 128)
  v  = x @ wv                      (token-major) + ones column per head
  per head pair (2fc, 2fc+1), per q-block Q (512 wide):
    s^T[kchunk] = kT_h^T @ qT_h    (K=64 contraction, row-tiled pair -> concurrent)
    p = exp(0.125 * s^T)  (ScalarE, bf16 out); causal-zeroed on GpSimd for
        diagonal chunks; fully-masked chunks skipped entirely.
    o^T[65,512] += v_aug_h^T @ p   (v_aug has a ones column -> row 64 = softmax
        denominators, fused into the same matmul)
    yT_h = o^T[0:64] * (1/o^T[64])  (PE K=1 broadcast of the reciprocal row)
  partial[T-block] = yT^T @ wp + 0.5*b_proj ; ReduceScatter(add) over the pair.
"""

import sys

if "/opt/trn_rl_repo" not in sys.path:
    sys.path.insert(0, "/opt/trn_rl_repo")

import numpy as np
import ml_dtypes

import concourse.bass as bass
import concourse.bacc as bacc
import concourse.mybir as mybir
import concourse.tile as tile
from concourse.bass import ts, ds
from concourse.bass_utils import run_bass_kernel_spmd

BF16 = ml_dtypes.bfloat16
N_CORES = 8
B, T, C = 4, 2048, 1024
NH, HS = 16, 64
H_LOC = NH // 2        # heads per core
F = H_LOC * HS         # 512 local qkv features
NFC = F // 128         # 4 feature chunks (one head pair each)
NKC = T // 128         # 16 key chunks
NQ = T // 512          # 4 query blocks
NCOL = C // 512        # 2 output column blocks
REPLICA_GROUPS = [[0, 1], [2, 3], [4, 5], [6, 7]]

FP32 = mybir.dt.float32
BF = mybir.dt.bfloat16


def _build_nc():
    # Bacc (not plain Bass): its compile() pipeline runs
    # generate_event_semaphores, which splits sync waits so no instruction
    # carries more than the hardware allows (walrus rejects >1 otherwise).
    nc = bacc.Bacc(None, target_bir_lowering=False, num_devices=N_CORES)

    xT = nc.dram_tensor("xT", [C, T], BF, kind="ExternalInput")
    wq = nc.dram_tensor("wq", [C, F], BF, kind="ExternalInput")
    wk = nc.dram_tensor("wk", [C, F], BF, kind="ExternalInput")
    wv = nc.dram_tensor("wv", [C, F], BF, kind="ExternalInput")
    bq = nc.dram_tensor("bq", [F], FP32, kind="ExternalInput")
    bk = nc.dram_tensor("bk", [F], FP32, kind="ExternalInput")
    bv = nc.dram_tensor("bv", [F], FP32, kind="ExternalInput")
    wp = nc.dram_tensor("wp", [F, C], BF, kind="ExternalInput")
    bp = nc.dram_tensor("bp", [C], FP32, kind="ExternalInput")
    out = nc.dram_tensor("out", [T // 2, C], FP32, kind="ExternalOutput")

    with tile.TileContext(nc) as tc:
        _body(tc, xT, wq, wk, wv, bq, bk, bv, wp, bp, out)
    nc.compile()
    return nc


def _body(tc, xT, wq, wk, wv, bq, bk, bv, wp, bp, out):
    nc = tc.nc
    import contextlib

    ctx = contextlib.ExitStack()
    with ctx:
        wpool = ctx.enter_context(tc.tile_pool(name="weights", bufs=1))
        apool = ctx.enter_context(tc.tile_pool(name="acts", bufs=1))
        ppool = ctx.enter_context(tc.tile_pool(name="ptiles", bufs=3))
        npool = ctx.enter_context(tc.tile_pool(name="norm", bufs=2))
        outp = ctx.enter_context(tc.tile_pool(name="outsb", bufs=3))
        ps_s = ctx.enter_context(tc.tile_pool(name="ps_s", bufs=2, space="PSUM"))
        ps_o = ctx.enter_context(tc.tile_pool(name="ps_o", bufs=2, space="PSUM"))
        dpool = ctx.enter_context(tc.tile_pool(name="dram", bufs=1, space="DRAM"))

        # ---- stage inputs into SBUF ----
        x_sb = wpool.tile([128, C // 128, T], BF)
        nc.sync.dma_start(out=x_sb, in_=xT.rearrange("(ko p) t -> p ko t", p=128))
        wq_sb = wpool.tile([128, C // 128, F], BF)
        nc.sync.dma_start(out=wq_sb, in_=wq.rearrange("(ko p) f -> p ko f", p=128))
        wk_sb = wpool.tile([128, C // 128, F], BF)
        nc.sync.dma_start(out=wk_sb, in_=wk.rearrange("(ko p) f -> p ko f", p=128))
        wv_sb = wpool.tile([128, C // 128, F], BF)
        nc.sync.dma_start(out=wv_sb, in_=wv.rearrange("(ko p) f -> p ko f", p=128))
        wp_sb = wpool.tile([128, NFC, C], BF)
        nc.sync.dma_start(out=wp_sb, in_=wp.rearrange("(ko p) n -> p ko n", p=128))

        bq_sb = wpool.tile([128, NFC], FP32)
        nc.sync.dma_start(out=bq_sb, in_=bq.rearrange("(fo p) -> p fo", p=128))
        bk_sb = wpool.tile([128, NFC], FP32)
        nc.sync.dma_start(out=bk_sb, in_=bk.rearrange("(fo p) -> p fo", p=128))
        # broadcast biases across partitions (for token-major layouts)
        bv_bc = wpool.tile([128, F], FP32)
        nc.sync.dma_start(
            out=bv_bc,
            in_=bass.AP(tensor=bv.ap().tensor, offset=0, ap=[[0, 128], [1, F]]),
        )
        bp_bc = wpool.tile([128, C], FP32)
        nc.sync.dma_start(
            out=bp_bc,
            in_=bass.AP(tensor=bp.ap().tensor, offset=0, ap=[[0, 128], [1, C]]),
        )

        # ---- persistent activations ----
        qT_sb = apool.tile([128, NFC, T], BF)   # q, feature-major
        kT_sb = apool.tile([128, NFC, T], BF)   # k, feature-major
        # v token-major, 66-stride per head: cols 0:64 = v, col 64 = ones
        v_sb = apool.tile([128, NKC, H_LOC, 66], BF)
        nc.vector.memset(v_sb[:, :, :, 64:65], 1.0)
        yT_sb = apool.tile([128, NFC, T], BF)   # attention out, feature-major

        partial = dpool.tile([T, C], FP32)      # c_proj partial (pre-reduce)
        # per-Q-block ReduceScatter halves: core keeps [256,1024] per block
        rs_outs = [dpool.tile([256, C], FP32, name=f"rs_out{q}") for q in range(NQ)]

        KO = C // 128  # 8 contraction chunks for the projections

        # ---- phase 1: qT, kT (feature-major) ----
        for name, w_sb, b_sb, dst in (("q", wq_sb, bq_sb, qT_sb), ("k", wk_sb, bk_sb, kT_sb)):
            for fc in range(NFC):
                for tq in range(NQ):
                    ps = ps_s.tile([128, 512], FP32, tag="sA")
                    for kc in range(KO):
                        nc.tensor.matmul(
                            ps,
                            lhsT=w_sb[:, kc, ts(fc, 128)],
                            rhs=x_sb[:, kc, ts(tq, 512)],
                            start=(kc == 0),
                            stop=(kc == KO - 1),
                        )
                    nc.scalar.activation(
                        out=dst[:, fc, ts(tq, 512)],
                        in_=ps,
                        func=mybir.ActivationFunctionType.Identity,
                        bias=b_sb[:, fc : fc + 1],
                        scale=1.0,
                    )

        # ---- phase 1b: v (token-major) ----
        for tc_i in range(NKC):
            ps = ps_s.tile([128, 512], FP32, tag="sB")
            for kc in range(KO):
                nc.tensor.matmul(
                    ps,
                    lhsT=x_sb[:, kc, ts(tc_i, 128)],
                    rhs=wv_sb[:, kc, :],
                    start=(kc == 0),
                    stop=(kc == KO - 1),
                )
            nc.vector.tensor_add(
                out=v_sb[:, tc_i, :, 0:64],
                in0=ps.rearrange("p (h f) -> p h f", h=H_LOC),
                in1=bv_bc.rearrange("p (h f) -> p h f", h=H_LOC),
            )

        # ---- phase 2+3: attention per q-block, then that block's c_proj ----
        for Q in range(NQ):
            nkc = 4 * Q + 4  # causal: only key chunks 0 .. 4Q+3 contribute
            for fc in range(NFC):  # head pair (2fc, 2fc+1)
                oA = ps_o.tile([65, 512], FP32, tag="oA")
                oB = ps_o.tile([65, 512], FP32, tag="oB")
                for kc in range(nkc):
                    diag = kc >= 4 * Q  # chunk crosses the causal boundary
                    sA = ps_s.tile([128, 512], FP32, tag="sA")
                    sB = ps_s.tile([128, 512], FP32, tag="sB")
                    nc.tensor.matmul(
                        sA,
                        lhsT=kT_sb[0:64, fc, ts(kc, 128)],
                        rhs=qT_sb[0:64, fc, ts(Q, 512)],
                        start=True,
                        stop=True,
                        tile_position=(0, 0),
                    )
                    nc.tensor.matmul(
                        sB,
                        lhsT=kT_sb[64:128, fc, ts(kc, 128)],
                        rhs=qT_sb[64:128, fc, ts(Q, 512)],
                        start=True,
                        stop=True,
                        tile_position=(64, 0),
                    )
                    pA = ppool.tile([128, 512], BF, tag="pA")
                    pB = ppool.tile([128, 512], BF, tag="pB")
                    nc.scalar.activation(
                        out=pA, in_=sA, func=mybir.ActivationFunctionType.Exp,
                        scale=0.125,
                    )
                    nc.scalar.activation(
                        out=pB, in_=sB, func=mybir.ActivationFunctionType.Exp,
                        scale=0.125,
                    )
                    if diag:
                        # zero entries with k_global > q_global:
                        # keep where 512*Q + qc - 128*kc - kr >= 0
                        for p_t in (pA, pB):
                            nc.gpsimd.affine_select(
                                out=p_t,
                                in_=p_t,
                                compare_op=mybir.AluOpType.is_ge,
                                fill=0.0,
                                base=512 * Q - 128 * kc,
                                channel_multiplier=-1,
                                pattern=[[1, 512]],
                            )
                    nc.tensor.matmul(
                        oA,
                        lhsT=v_sb[:, kc, 2 * fc, 0:65],
                        rhs=pA,
                        start=(kc == 0),
                        stop=(kc == nkc - 1),
                    )
                    nc.tensor.matmul(
                        oB,
                        lhsT=v_sb[:, kc, 2 * fc + 1, 0:65],
                        rhs=pB,
                        start=(kc == 0),
                        stop=(kc == nkc - 1),
                    )
                # normalize: yT_h = oT[0:64] * (1 / oT[64]).
                # Everything off the TensorEngine queue: DVE approx
                # reciprocal + DMA partition-broadcast + DVE multiply.
                oA_sb = npool.tile([65, 512], FP32, tag="oAsb")
                oB_sb = npool.tile([65, 512], FP32, tag="oBsb")
                nc.vector.tensor_copy(out=oA_sb, in_=oA)
                nc.vector.tensor_copy(out=oB_sb, in_=oB)
                # custom-DVE reciprocal_approx_fast mishandles inputs at a
                # nonzero partition base -- stage row 64 down to partition 0
                rzA = npool.tile([1, 512], FP32, tag="rzA")
                rzB = npool.tile([1, 512], FP32, tag="rzB")
                nc.vector.tensor_copy(out=rzA, in_=oA_sb[64:65, :])
                nc.vector.tensor_copy(out=rzB, in_=oB_sb[64:65, :])
                rA = npool.tile([1, 512], FP32, tag="rA")
                rB = npool.tile([1, 512], FP32, tag="rB")
                nc.vector.reciprocal_approx_fast(out=rA, in_=rzA)
                nc.vector.reciprocal_approx_fast(out=rB, in_=rzB)
                # partition-broadcast via DRAM bounce (SBUF APs need nonzero
                # partition step; DRAM APs don't)
                rAd = dpool.tile([512], FP32, tag="rAd", bufs=2)
                rBd = dpool.tile([512], FP32, tag="rBd", bufs=2)
                nc.sync.dma_start(out=rAd[None, :], in_=rA)
                nc.sync.dma_start(out=rBd[None, :], in_=rB)
                bcA = npool.tile([64, 512], FP32, tag="bcA")
                bcB = npool.tile([64, 512], FP32, tag="bcB")
                nc.sync.dma_start(
                    out=bcA,
                    in_=bass.AP(tensor=rAd.tensor, offset=rAd.offset, ap=[[0, 64], [1, 512]]),
                )
                nc.sync.dma_start(
                    out=bcB,
                    in_=bass.AP(tensor=rBd.tensor, offset=rBd.offset, ap=[[0, 64], [1, 512]]),
                )
                # head A lives on partitions 0:64 of chunk fc
                nc.vector.tensor_mul(
                    out=yT_sb[0:64, fc, ts(Q, 512)], in0=oA_sb[0:64, :], in1=bcA
                )
                # head B must land on partitions 64:128 -> stage + DMA shift
                yB = npool.tile([64, 512], BF, tag="yB")
                nc.vector.tensor_mul(out=yB, in0=oB_sb[0:64, :], in1=bcB)
                nc.sync.dma_start(out=yT_sb[64:128, fc, ts(Q, 512)], in_=yB)

            # c_proj for this block of 512 tokens
            for tb in range(4):
                trow = Q * 4 + tb
                for ncol in range(NCOL):
                    ps = ps_s.tile([128, 512], FP32, tag="sA")
                    for fc in range(NFC):
                        nc.tensor.matmul(
                            ps,
                            lhsT=yT_sb[:, fc, ts(trow, 128)],
                            rhs=wp_sb[:, fc, ts(ncol, 512)],
                            start=(fc == 0),
                            stop=(fc == NFC - 1),
                        )
                    o_sb = outp.tile([128, 512], FP32, tag="osb")
                    nc.vector.tensor_add(out=o_sb, in0=ps, in1=bp_bc[:, ts(ncol, 512)])
                    nc.sync.dma_start(
                        out=partial[ds(trow * 128, 128), ts(ncol, 512)], in_=o_sb
                    )

            # reduce this 512-token block across the batch pair while later
            # blocks still compute; each core keeps 256 of the 512 rows.
            nc.gpsimd.collective_compute(
                "ReduceScatter",
                mybir.AluOpType.add,
                replica_groups=REPLICA_GROUPS,
                ins=[partial[ds(Q * 512, 512), :]],
                outs=[rs_outs[Q][:]],
            )
            nc.sync.dma_start(
                out=out.ap()[ds(Q * 256, 256), :], in_=rs_outs[Q][:]
            )


_NC_CACHE = None


def _get_nc():
    global _NC_CACHE
    if _NC_CACHE is None:
        _NC_CACHE = _build_nc()
    return _NC_CACHE


def kernel(x, w_attn, b_attn, w_proj, b_proj):
    x = np.asarray(x)
    w_attn = np.asarray(w_attn)
    b_attn = np.asarray(b_attn)
    w_proj = np.asarray(w_proj)
    b_proj = np.asarray(b_proj)

    nc = _get_nc()

    in_maps = []
    for i in range(N_CORES):
        b, g = i // 2, i % 2
        cols = slice(g * F, (g + 1) * F)
        in_maps.append(
            {
                "xT": np.ascontiguousarray(x[b].T).astype(BF16),
                "wq": np.ascontiguousarray(w_attn[:, g * F : (g + 1) * F]).astype(BF16),
                "wk": np.ascontiguousarray(
                    w_attn[:, C + g * F : C + (g + 1) * F]
                ).astype(BF16),
                "wv": np.ascontiguousarray(
                    w_attn[:, 2 * C + g * F : 2 * C + (g + 1) * F]
                ).astype(BF16),
                "bq": np.ascontiguousarray(b_attn[g * F : (g + 1) * F]).astype(
                    np.float32
                ),
                "bk": np.ascontiguousarray(b_attn[C + g * F : C + (g + 1) * F]).astype(
                    np.float32
                ),
                "bv": np.ascontiguousarray(
                    b_attn[2 * C + g * F : 2 * C + (g + 1) * F]
                ).astype(np.float32),
                "wp": np.ascontiguousarray(w_proj[g * F : (g + 1) * F, :]).astype(BF16),
                "bp": (b_proj * 0.5).astype(np.float32),
            }
        )

    global _last_in_maps
    _last_in_maps = in_maps  # stashed for external profiling harnesses
    res = run_bass_kernel_spmd(nc, in_maps, core_ids=list(range(N_CORES)))

    # Each core's "out" holds NQ blocks of 256 rows: block Q is the core's
    # ReduceScatter half of token rows [Q*512, (Q+1)*512) -- rank 0 (even
    # core) the first 256, rank 1 (odd core) the last 256.
    out = np.empty((B, T, C), dtype=np.float32)
    for b in range(B):
        even = res.results[2 * b]["out"].reshape(NQ, 256, C)
        odd = res.results[2 * b + 1]["out"].reshape(NQ, 256, C)
        blocks = out[b].reshape(NQ, 2, 256, C)
        blocks[:, 0] = even
        blocks[:, 1] = odd
    return out
